# revision 1
# baseline (speedup 1.0000x reference)
"""PixPro loss kernel for 8 Trainium2 NeuronCores.

Data-parallel over batch: 1024 samples -> 128 per core (= SBUF partitions).

Heavy part (cos similarity over 512 channels x 49 grid points):
  host transposes per-core features to [B=128, N=49, C=512] (c contiguous);
  samples stay on SBUF partitions. Per grid point n, a single fused
  scalar_tensor_tensor with accum_out computes the channel reduction in one
  pass: dot (b*m), |b|^2, |m|^2 each via one DVE pass. No PE, no PSUM.
  Feature DMAs are triggered from the idle sync (SP) queue for lookahead.

Mask part (grids / distances / masks) runs with samples on partitions and
overlaps the feature DMAs. Per-core output is [128, 2] = (masked loss sum
contribution, intersection flag); host does the final psum + divide.
"""

import sys

import numpy as np

if "/opt/trn_rl_repo" not in sys.path:
    sys.path.insert(0, "/opt/trn_rl_repo")

B = 1024
C = 512
S = 7
N = S * S  # 49
NCORES = 8
BP = B // NCORES  # 128 samples per core
NBLK = 7  # n-blocks of 7 grid points each
IO_BUFS = 4
EPS = 1e-6
THRESH2 = 0.7 * 0.7

_t = np.linspace(0.0, 1.0, S).astype(np.float32)
_n = np.arange(N)
TX_TAB = np.ascontiguousarray(np.tile(_t[_n // S], (BP, 1)))  # [128, 49]
TY_TAB = np.ascontiguousarray(np.tile(_t[_n % S], (BP, 1)))  # [128, 49]

ALL_PARTS = frozenset(("mask", "heavy", "tail"))

_NC = None


def _emit(tc, d, parts=ALL_PARTS):
    """Emit the tile kernel. d: dict of DRAM APs."""
    from contextlib import ExitStack

    from concourse import mybir

    nc = tc.nc
    f32 = mybir.dt.float32
    A = mybir.AluOpType
    AX = mybir.AxisListType

    with ExitStack() as ctx:
        pers = ctx.enter_context(tc.tile_pool(name="pers", bufs=1))
        io = ctx.enter_context(tc.tile_pool(name="io", bufs=IO_BUFS))
        work = ctx.enter_context(tc.tile_pool(name="work", bufs=1))

        # ---- persistent small tiles ----
        pb_t = pers.tile([BP, 4], f32, tag="pb_t")
        pm_t = pers.tile([BP, 4], f32, tag="pm_t")
        fb_t = pers.tile([BP, 1], f32, tag="fb_t")
        fm_t = pers.tile([BP, 1], f32, tag="fm_t")
        tx_t = pers.tile([BP, N], f32, tag="tx_t")
        ty_t = pers.tile([BP, N], f32, tag="ty_t")

        nc.gpsimd.dma_start(pb_t[:], d["pb"][:])
        nc.gpsimd.dma_start(pm_t[:], d["pm"][:])
        nc.gpsimd.dma_start(fb_t[:], d["fb"][:])
        nc.gpsimd.dma_start(fm_t[:], d["fm"][:])
        nc.gpsimd.dma_start(tx_t[:], d["tx"][:])
        nc.gpsimd.dma_start(ty_t[:], d["ty"][:])

        xb = pb_t[:, 0:1]
        yb = pb_t[:, 1:2]
        wb = pb_t[:, 2:3]
        hb = pb_t[:, 3:4]
        xm = pm_t[:, 0:1]
        ym = pm_t[:, 1:2]
        wm = pm_t[:, 2:3]
        hm = pm_t[:, 3:4]

        out_sb = pers.tile([BP, 2], f32, tag="out_sb")

        if "mask" in parts:
            # ---- mask part (samples on partitions) ----
            # flip: y' = y + h*f, h' = h*(1 - 2f)
            yb2 = pers.tile([BP, 1], f32, tag="yb2")
            hb2 = pers.tile([BP, 1], f32, tag="hb2")
            ym2 = pers.tile([BP, 1], f32, tag="ym2")
            hm2 = pers.tile([BP, 1], f32, tag="hm2")
            tmp1 = pers.tile([BP, 1], f32, tag="tmp1")
            nc.vector.scalar_tensor_tensor(yb2[:], fb_t[:], hb, yb, A.mult, A.add)
            nc.vector.tensor_scalar(tmp1[:], fb_t[:], -2.0, 1.0, A.mult, A.add)
            nc.vector.tensor_tensor(hb2[:], tmp1[:], hb, A.mult)
            nc.vector.scalar_tensor_tensor(ym2[:], fm_t[:], hm, ym, A.mult, A.add)
            nc.vector.tensor_scalar(tmp1[:], fm_t[:], -2.0, 1.0, A.mult, A.add)
            nc.vector.tensor_tensor(hm2[:], tmp1[:], hm, A.mult)

            # grids [BP, N]
            gxb = pers.tile([BP, N], f32, tag="gxb")
            gyb = pers.tile([BP, N], f32, tag="gyb")
            gxm = pers.tile([BP, N], f32, tag="gxm")
            gym = pers.tile([BP, N], f32, tag="gym")
            nc.vector.tensor_scalar(gxb[:], tx_t[:], wb, xb, A.mult, A.add)
            nc.vector.tensor_scalar(
                gyb[:], ty_t[:], hb2[:, 0:1], yb2[:, 0:1], A.mult, A.add
            )
            nc.vector.tensor_scalar(gxm[:], tx_t[:], wm, xm, A.mult, A.add)
            nc.vector.tensor_scalar(
                gym[:], ty_t[:], hm2[:, 0:1], ym2[:, 0:1], A.mult, A.add
            )

            # tau^2 = 0.49 * (w^2 + h^2) per side
            tau2b = pers.tile([BP, 1], f32, tag="tau2b")
            tau2m = pers.tile([BP, 1], f32, tag="tau2m")
            nc.vector.tensor_tensor(tmp1[:], wb, wb, A.mult)
            nc.vector.scalar_tensor_tensor(tau2b[:], hb, hb, tmp1[:], A.mult, A.add)
            nc.vector.tensor_scalar_mul(tau2b[:], tau2b[:], THRESH2)
            nc.vector.tensor_tensor(tmp1[:], wm, wm, A.mult)
            nc.vector.scalar_tensor_tensor(tau2m[:], hm, hm, tmp1[:], A.mult, A.add)
            nc.vector.tensor_scalar_mul(tau2m[:], tau2m[:], THRESH2)

            # D2[p, i, j] = (gxb_i - gxm_j)^2 + (gyb_i - gym_j)^2
            t0 = pers.tile([BP, N, N], f32, tag="t0")
            t1 = pers.tile([BP, N, N], f32, tag="t1")
            t2 = pers.tile([BP, N, N], f32, tag="t2")
            gxb_i = gxb[:].unsqueeze(2).broadcast_to([BP, N, N])
            gxm_j = gxm[:].unsqueeze(1).broadcast_to([BP, N, N])
            gyb_i = gyb[:].unsqueeze(2).broadcast_to([BP, N, N])
            gym_j = gym[:].unsqueeze(1).broadcast_to([BP, N, N])
            nc.vector.tensor_tensor(t0[:], gxb_i, gxm_j, A.subtract)  # dx
            nc.vector.tensor_tensor(t1[:], gyb_i, gym_j, A.subtract)  # dy
            nc.vector.tensor_tensor(t2[:], t0[:], t0[:], A.mult)  # dx^2
            nc.scalar.square(t0[:], t1[:])  # dy^2 (ACT)
            nc.vector.tensor_tensor(t1[:], t2[:], t0[:], A.add)  # D2 -> t1

            # masks + counts + mask marginals
            nnzb = pers.tile([BP, 1], f32, tag="nnzb")
            nnzm = pers.tile([BP, 1], f32, tag="nnzm")
            colsum_b = pers.tile([BP, N], f32, tag="colsum_b")  # sum_i mask_b[i, j]
            rowsum_m = pers.tile([BP, N], f32, tag="rowsum_m")  # sum_j mask_m[i, j]
            nc.vector.tensor_scalar(
                t2[:], t1[:], tau2b[:, 0:1], None, A.is_lt, op1=A.add,
                accum_out=nnzb[:],
            )
            nc.vector.tensor_reduce(
                colsum_b[:], t2[:].transpose([0, 2, 1]), AX.X, A.add
            )
            nc.vector.tensor_scalar(
                t2[:], t1[:], tau2m[:, 0:1], None, A.is_lt, op1=A.add,
                accum_out=nnzm[:],
            )
            nc.vector.tensor_reduce(rowsum_m[:], t2[:], AX.X, A.add)

            # intersection flag: (2|cx1-cx2| < wb+wm) & (2|cy1-cy2| < hb+hm)
            # (uses raw p_base/p_moment, not flipped)
            u1 = pers.tile([BP, 1], f32, tag="u1")
            u2 = pers.tile([BP, 1], f32, tag="u2")
            okx = pers.tile([BP, 1], f32, tag="okx")
            oky = pers.tile([BP, 1], f32, tag="oky")
            inter = pers.tile([BP, 1], f32, tag="inter")
            nc.vector.scalar_tensor_tensor(u1[:], wb, 0.5, xb, A.mult, A.add)
            nc.vector.scalar_tensor_tensor(u2[:], wm, 0.5, xm, A.mult, A.add)
            nc.vector.tensor_tensor(u1[:], u1[:], u2[:], A.subtract)
            nc.scalar.activation(u1[:], u1[:], mybir.ActivationFunctionType.Abs)
            nc.vector.tensor_tensor(u2[:], wb, wm, A.add)
            nc.vector.scalar_tensor_tensor(okx[:], u1[:], 2.0, u2[:], A.mult, A.is_lt)
            nc.vector.scalar_tensor_tensor(u1[:], hb, 0.5, yb, A.mult, A.add)
            nc.vector.scalar_tensor_tensor(u2[:], hm, 0.5, ym, A.mult, A.add)
            nc.vector.tensor_tensor(u1[:], u1[:], u2[:], A.subtract)
            nc.scalar.activation(u1[:], u1[:], mybir.ActivationFunctionType.Abs)
            nc.vector.tensor_tensor(u2[:], hb, hm, A.add)
            nc.vector.scalar_tensor_tensor(oky[:], u1[:], 2.0, u2[:], A.mult, A.is_lt)
            nc.vector.tensor_tensor(inter[:], okx[:], oky[:], A.mult)

        if "heavy" in parts:
            # ---- heavy part: fused multiply+channel-reduce per grid point ----
            dot_sb = pers.tile([BP, N], f32, tag="dot_sb")
            nrm_b = pers.tile([BP, N], f32, tag="nrm_b")
            nrm_m = pers.tile([BP, N], f32, tag="nrm_m")
            scr_d = work.tile([BP, C], f32, tag="scr_d")
            scr_a = work.tile([BP, C], f32, tag="scr_a")
            nblk = N // NBLK
            for blk in range(NBLK):
                n0 = blk * nblk
                b_t = io.tile([BP, nblk, C], f32, tag="b_t")
                m_t = io.tile([BP, nblk, C], f32, tag="m_t")
                h = nblk // 2
                nc.sync.dma_start(b_t[:, :h, :], d["bt"][:, n0 : n0 + h, :])
                nc.sync.dma_start(m_t[:, :h, :], d["mt"][:, n0 : n0 + h, :])
                nc.sync.dma_start(b_t[:, h:, :], d["bt"][:, n0 + h : n0 + nblk, :])
                nc.sync.dma_start(m_t[:, h:, :], d["mt"][:, n0 + h : n0 + nblk, :])
                for j in range(nblk):
                    n = n0 + j
                    # dot on DVE (fused multiply + channel accum)
                    nc.vector.scalar_tensor_tensor(
                        scr_d[:], b_t[:, j, :], 1.0, m_t[:, j, :],
                        A.mult, A.mult, accum_out=dot_sb[:, n : n + 1],
                    )
                    # |b|^2 on ACT (fused square + channel accum)
                    nc.scalar.activation(
                        scr_a[:], b_t[:, j, :],
                        mybir.ActivationFunctionType.Square,
                        accum_out=nrm_b[:, n : n + 1],
                    )
                    # |m|^2: split between DVE and ACT to balance engines
                    if n % 3 == 0:
                        nc.vector.scalar_tensor_tensor(
                            scr_d[:], m_t[:, j, :], 1.0, m_t[:, j, :],
                            A.mult, A.mult, accum_out=nrm_m[:, n : n + 1],
                        )
                    else:
                        nc.scalar.activation(
                            scr_a[:], m_t[:, j, :],
                            mybir.ActivationFunctionType.Square,
                            accum_out=nrm_m[:, n : n + 1],
                        )

        do_ttr = "tail" in parts or "ttr" in parts
        do_cos = do_ttr or "cos" in parts
        if do_cos:
            # ---- cos assembly ----
            den = pers.tile([BP, N], f32, tag="den")
            cos_t = pers.tile([BP, N], f32, tag="cos_t")
            nc.scalar.sqrt(nrm_b[:], nrm_b[:])
            nc.scalar.sqrt(nrm_m[:], nrm_m[:])
            nc.vector.tensor_scalar_max(nrm_b[:], nrm_b[:], EPS)
            nc.vector.tensor_scalar_max(nrm_m[:], nrm_m[:], EPS)
            nc.vector.tensor_tensor(den[:], nrm_b[:], nrm_m[:], A.mult)
            nc.vector.reciprocal(den[:], den[:])
            nc.vector.tensor_tensor(cos_t[:], dot_sb[:], den[:], A.mult)

        if do_ttr:
            # s_b = sum_j cos[j]*colsum_b[j]; s_m = sum_i cos[i]*rowsum_m[i]
            sb_s = pers.tile([BP, 1], f32, tag="sb_s")
            sm_s = pers.tile([BP, 1], f32, tag="sm_s")
            scr = pers.tile([BP, N], f32, tag="scr")
            nc.vector.tensor_tensor(scr[:], cos_t[:], colsum_b[:], A.mult)
            nc.vector.tensor_reduce(sb_s[:], scr[:], AX.X, A.add)
            nc.vector.tensor_tensor(scr[:], cos_t[:], rowsum_m[:], A.mult)
            nc.vector.tensor_reduce(sm_s[:], scr[:], AX.X, A.add)

        if "tail" in parts:
            # loss = s / max(nnz, 1) per side; contribution = (lb+lm)*inter
            lb = pers.tile([BP, 1], f32, tag="lb")
            lm = pers.tile([BP, 1], f32, tag="lm")
            nc.vector.tensor_scalar_max(nnzb[:], nnzb[:], 1.0)
            nc.vector.tensor_scalar_max(nnzm[:], nnzm[:], 1.0)
            nc.vector.reciprocal(nnzb[:], nnzb[:])
            nc.vector.reciprocal(nnzm[:], nnzm[:])
            nc.vector.tensor_tensor(lb[:], sb_s[:], nnzb[:], A.mult)
            nc.vector.tensor_tensor(lm[:], sm_s[:], nnzm[:], A.mult)
            nc.vector.tensor_tensor(lb[:], lb[:], lm[:], A.add)
            nc.vector.tensor_tensor(lb[:], lb[:], inter[:], A.mult)

            nc.vector.tensor_copy(out_sb[:, 0:1], lb[:])
            nc.vector.tensor_copy(out_sb[:, 1:2], inter[:])
        elif do_ttr:
            nc.vector.tensor_copy(out_sb[:, 0:1], sb_s[:])
            nc.vector.tensor_copy(out_sb[:, 1:2], sm_s[:])
        elif do_cos:
            nc.vector.tensor_copy(out_sb[:, 0:1], cos_t[:, 0:1])
            nc.vector.tensor_copy(out_sb[:, 1:2], den[:, 0:1])
        elif "mask" in parts:
            nc.vector.tensor_copy(out_sb[:, 0:1], nnzb[:])
            nc.vector.tensor_copy(out_sb[:, 1:2], inter[:])
        elif "heavy" in parts:
            nc.vector.tensor_copy(out_sb[:, 0:1], dot_sb[:, 0:1])
            nc.vector.tensor_copy(out_sb[:, 1:2], nrm_b[:, 0:1])
        else:
            nc.vector.tensor_copy(out_sb[:, 0:2], pb_t[:, 0:2])

        nc.gpsimd.dma_start(d["o"][:], out_sb[:])


def build(debug=False, parts=ALL_PARTS):
    import concourse.bacc as bacc
    import concourse.tile as tile
    from concourse import mybir

    nc = bacc.Bacc(
        "TRN2",
        target_bir_lowering=False,
        debug=debug,
        enable_asserts=False,
        num_devices=NCORES,
    )
    f32 = mybir.dt.float32
    d = {
        "bt": nc.dram_tensor("bt", [BP, N, C], f32, kind="ExternalInput").ap(),
        "mt": nc.dram_tensor("mt", [BP, N, C], f32, kind="ExternalInput").ap(),
        "pb": nc.dram_tensor("pb", [BP, 4], f32, kind="ExternalInput").ap(),
        "pm": nc.dram_tensor("pm", [BP, 4], f32, kind="ExternalInput").ap(),
        "fb": nc.dram_tensor("fb", [BP, 1], f32, kind="ExternalInput").ap(),
        "fm": nc.dram_tensor("fm", [BP, 1], f32, kind="ExternalInput").ap(),
        "tx": nc.dram_tensor("tx", [BP, N], f32, kind="ExternalInput").ap(),
        "ty": nc.dram_tensor("ty", [BP, N], f32, kind="ExternalInput").ap(),
        "o": nc.dram_tensor("o", [BP, 2], f32, kind="ExternalOutput").ap(),
    }
    with tile.TileContext(nc) as tc:
        _emit(tc, d, parts)
    nc.compile()
    return nc


def make_in_maps(base, moment, p_base, p_moment, f_base, f_moment):
    in_maps = []
    for k in range(NCORES):
        sl = slice(k * BP, (k + 1) * BP)
        bt = np.ascontiguousarray(
            np.asarray(base[sl], dtype=np.float32).reshape(BP, C, N).transpose(0, 2, 1)
        )
        mt = np.ascontiguousarray(
            np.asarray(moment[sl], dtype=np.float32)
            .reshape(BP, C, N)
            .transpose(0, 2, 1)
        )
        in_maps.append(
            {
                "bt": bt,
                "mt": mt,
                "pb": np.ascontiguousarray(np.asarray(p_base[sl], dtype=np.float32)),
                "pm": np.ascontiguousarray(np.asarray(p_moment[sl], dtype=np.float32)),
                "fb": np.ascontiguousarray(np.asarray(f_base[sl], dtype=np.float32)),
                "fm": np.ascontiguousarray(np.asarray(f_moment[sl], dtype=np.float32)),
                "tx": TX_TAB,
                "ty": TY_TAB,
            }
        )
    return in_maps


def reduce_outputs(per_core_outs):
    """per_core_outs: list of [128, 2] arrays -> final scalar loss."""
    allo = np.concatenate([np.asarray(o, dtype=np.float64) for o in per_core_outs])
    pos = allo[:, 0].sum()
    cnt = allo[:, 1].sum()
    return np.asarray(-pos / max(cnt, 1.0), dtype=np.float32)


def kernel(base, moment, p_base, p_moment, f_base, f_moment, _trace=False):
    global _NC
    from concourse.bass_utils import run_bass_kernel_spmd

    if _NC is None:
        _NC = build()
    in_maps = make_in_maps(base, moment, p_base, p_moment, f_base, f_moment)
    res = run_bass_kernel_spmd(_NC, in_maps, core_ids=list(range(NCORES)), trace=_trace)
    out = reduce_outputs([r["o"] for r in res.results])
    if _trace:
        return out, res
    return out

